# revision 11
# baseline (speedup 1.0000x reference)
"""ColumnParallelLinear + per-token LoRA (punica add_lora) on 8 NeuronCores.

out = x @ W^T + b + B[idx] @ (A[idx] @ x^T), idx==-1 skips LoRA.

Sharding: tensor-parallel over the output dim (vLLM ColumnParallelLinear):
weight, bias and lora_b are sharded 512-wide per core; lora_a and indices
are replicated. The per-token LoRA shrink (s = A @ x) is sharded over
tokens (256/core) and shared via an on-chip AllGather; the LoRA expand is
folded in as a dense matmul against the routing-masked shrink
(s_masked[t, (l,r)] = (idx[t]==l) * s[t, (l,r)]), so no gather/scatter of
B is ever needed.

Perf notes (measured on HW):
- The kernel is PE-throughput bound: the trace shows matmuls issuing
  back-to-back at the power-throttled clock (HAM k=13/16 ~ 1.95 GHz).
  So v2 cuts PE cycles with fp8e4m3 DoubleRow matmuls (2 MACs/cell/cyc):
  * base matmul: hybrid split — kb blocks 0..15 of the H=4096 contraction
    run as fp8 DoubleRow pairs, blocks 16..31 stay bf16. Full-fp8 misses
    the 2e-2 gate (rel 0.024 measured vs reference); the 16/32 hybrid
    lands at rel ~0.018.
  * LoRA expand: full fp8 DoubleRow (error contribution negligible).
- Scales keep every fp8 operand in e4m3's sweet spot AND make the fp8 and
  bf16 partial products accumulate at a common scale: x ships as 16*x
  (both halves), W as 64*W, A as 16*A, B as 64*B. Base psum = 1024*x@W^T,
  descaled by the copy-out scalar_tensor_tensor (mult 2^-10, add bias).
  Shrink psum = 16*s; expand psum = 1024*delta.
- Inputs stream in a few large chunked DMAs from host-side PE-tile-major
  layouts ([128 partitions, kb, free]), spread across the sync/scalar/
  vector queues (one queue runs its transfers serially at ~140 GB/s).
- The shrink also runs DoubleRow (pairs over kb), and goes first: its
  3.1MB working set is the smallest, and the AllGather (measured ~35us
  on this axon/ncfw path for 128KB/rank) gets the whole base phase to
  complete before tail(0) consumes it two groups later.
- LoRA-expand tails load each group's s_masked with two 128KB DMAs
  (prefetched 4 tails deep) and store per-128-token-block so the last
  group's copy-out pipelines with its DMA.
"""
import json

import numpy as np
import ml_dtypes

import concourse.bass as bass
import concourse.mybir as mybir
import concourse.tile as tile
from concourse.bass_utils import run_bass_kernel_spmd

T, H, O, L, R = 2048, 4096, 4096, 32, 16
N_CORES = 8
O_SH = O // N_CORES          # 512  output cols per core
T_LOC = T // N_CORES         # 256  tokens whose LoRA-shrink this core computes
KB = H // 128                # 32   contraction blocks
KBF = 16                     # kb blocks 0..KBF-1 run fp8 DoubleRow
KBB = KB - KBF               # kb blocks KBF..KB-1 stay bf16
LR = L * R                   # 512  stacked (lora, rank) rows
BF16 = mybir.dt.bfloat16
F32 = mybir.dt.float32
F8 = mybir.dt.float8e4
DR = mybir.MatmulPerfMode.DoubleRow
X_SCALE = 16.0               # x ships as 16*x (fp8 and bf16 halves)
W_SCALE = 64.0               # weight ships as 64*W
A_SCALE = 16.0               # lora_a ships as 16*A  -> shrink psum = 16*s
B_SCALE = 64.0               # lora_b ships as 64*B  -> expand psum = 1024*delta
INV_BASE = 1.0 / (X_SCALE * W_SCALE)


def _split_waits(raw: bytes) -> bytes:
    """This walrus build rejects instructions carrying more than one sync
    wait ("Too many sync wait commands"), but Tile attaches one wait per
    producing proc. Hoist all but one wait of each instruction onto
    single-wait NoOps inserted just before it on the same engine — the
    engine executes its stream in order, so the gating is identical."""
    m = json.loads(raw)
    ctr = 0
    for f in m["functions"]:
        for b in f["blocks"]:
            out = []
            for inst in b["instructions"]:
                si = inst.get("sync_info")
                waits = si.get("on_wait") if si else None
                if waits and len(waits) > 1:
                    for w in waits[:-1]:
                        ctr += 1
                        out.append({
                            "debug": inst.get("debug", 0),
                            "engine": inst["engine"],
                            "ins": [],
                            "name": f"I-wsplit-{ctr}",
                            "opcode": "NoOp",
                            "outs": [],
                            "sync_info": {"on_update": [], "on_wait": [w]},
                        })
                    si["on_wait"] = [waits[-1]]
                out.append(inst)
            b["instructions"] = out
    return json.dumps(m).encode()


class _WaitSplitBass(bass.Bass):
    def to_json_bytes(self) -> bytes:
        return _split_waits(super().to_json_bytes())


def _build() -> bass.Bass:
    nc = _WaitSplitBass()
    # all streamed inputs are PE-tile-major: [128 h-partitions, kb, free]
    x8Tr = nc.dram_tensor("x8Tr", [128, KBF, T], F8, kind="ExternalInput")
    xbTr = nc.dram_tensor("xbTr", [128, KBB, T], BF16, kind="ExternalInput")
    w8Tr = nc.dram_tensor("w8Tr", [128, KBF, O_SH], F8, kind="ExternalInput")
    wbTr = nc.dram_tensor("wbTr", [128, KBB, O_SH], BF16, kind="ExternalInput")
    xl_r = nc.dram_tensor("xl_r", [128, KB, T_LOC], F8, kind="ExternalInput")
    aTr = nc.dram_tensor("aTr", [128, KB, LR], F8, kind="ExternalInput")
    bTr = nc.dram_tensor("bTr", [128, 4, O_SH], F8, kind="ExternalInput")
    bias_row = nc.dram_tensor("bias_row", [1, O_SH], BF16, kind="ExternalInput")
    idx_bc_d = nc.dram_tensor("idx_bc", [128, T_LOC], F32, kind="ExternalInput")
    lrow_d = nc.dram_tensor("lrow", [128, 4], F32, kind="ExternalInput")
    out = nc.dram_tensor("out", [T, O_SH], F32, kind="ExternalOutput")

    with tile.TileContext(nc) as tc:
        with (
            tc.tile_pool(name="res", bufs=1) as res,          # long-lived SBUF
            tc.tile_pool(name="stream", bufs=4) as stream,    # streamed SBUF
            tc.tile_pool(name="ps", bufs=2, space="PSUM") as ps,
            tc.tile_pool(name="dram", bufs=1, space="DRAM") as dram,
        ):
            # ---------------- resident weight tiles ------------------------
            wt8 = res.tile([128, KBF, O_SH], F8, name="wt8")
            wtb = res.tile([128, KBB, O_SH], BF16, name="wtb")
            at_all = res.tile([128, KB, LR], F8, name="at_all")
            xl_all = res.tile([128, KB, T_LOC], F8, name="xl_all")
            bt_all = res.tile([128, 4, O_SH], F8, name="bt_all")
            base_sb = res.tile([128, 16 * O_SH], F32, name="base_sb")

            # bias broadcast [128, 512] f32 via K=1 ones-matmul; the psum
            # descale happens in the same copy-out op later.
            bias_r = res.tile([1, O_SH], BF16, name="bias_r")
            nc.scalar.dma_start(bias_r[:], bias_row[:])
            ones_t = res.tile([1, 128], BF16, name="ones_t")
            nc.vector.memset(ones_t[:], 1.0)

            # ---------------- Phase A: LoRA shrink for local tokens --------
            # One queue moves ~140 GB/s, so the 2.1MB at tensor alternates
            # chunks between scalar and gpsimd while xl heads the sync queue
            # — the shrink (300+ GB/s consumption) then barely starves.
            for c in range(8):
                kc = KB // 8
                eng = nc.scalar if c % 2 == 0 else nc.gpsimd
                eng.dma_start(
                    at_all[:, c * kc:(c + 1) * kc, :],
                    aTr[:, c * kc:(c + 1) * kc, :],
                )
                nc.sync.dma_start(
                    xl_all[:, c * kc:(c + 1) * kc, :],
                    xl_r[:, c * kc:(c + 1) * kc, :],
                )
            idx_bc = res.tile([128, T_LOC], F32, name="idx_bc_t")
            nc.scalar.dma_start(idx_bc[:], idx_bc_d[:])
            lrow = res.tile([128, 4], F32, name="lrow_t")
            nc.scalar.dma_start(lrow[:], lrow_d[:])

            bias_ps = ps.tile([128, O_SH], F32, name="bias_ps", tag="pso0")
            nc.tensor.matmul(bias_ps[:], ones_t[:], bias_r[:], start=True,
                             stop=True)
            bias_bc = res.tile([128, O_SH], F32, name="bias_bc")
            nc.vector.tensor_copy(bias_bc[:], bias_ps[:])

            ps_s = [ps.tile([128, T_LOC], F32, name=f"ps_s{m}", tag=f"pso{m}")
                    for m in range(4)]
            for kp in range(KB // 2):
                for m in range(4):
                    nc.tensor.matmul(
                        ps_s[m][:],
                        at_all[:, 2 * kp:2 * kp + 2, m * 128:(m + 1) * 128],
                        xl_all[:, 2 * kp:2 * kp + 2, :],
                        start=(kp == 0),
                        stop=(kp == KB // 2 - 1),
                        perf_mode=DR,
                    )

            # routing mask + fp8 downcast, fused: sm = (idx==l(p)) * 16*s
            cc_in = dram.tile([LR, T_LOC], F8, name="cc_in")
            sm = stream.tile([128, 4 * T_LOC], F8, name="sm", tag="sm")
            for m in range(4):
                nc.vector.scalar_tensor_tensor(
                    sm[:, m * T_LOC:(m + 1) * T_LOC],
                    idx_bc[:],
                    lrow[:, m:m + 1],
                    ps_s[m][:],
                    op0=mybir.AluOpType.is_equal,
                    op1=mybir.AluOpType.mult,
                )
            nc.gpsimd.dma_start(
                cc_in[:].rearrange("(m p) t -> p m t", p=128),
                sm[:].rearrange("p (m t) -> p m t", t=T_LOC),
            )

            cc_out = dram.tile([N_CORES, LR, T_LOC], F8, name="cc_out",
                               addr_space="Shared")
            nc.gpsimd.collective_compute(
                "AllGather",
                mybir.AluOpType.bypass,
                replica_groups=[list(range(N_CORES))],
                ins=[cc_in.opt()],
                outs=[cc_out.opt()],
            )

            # ---------------- Phase B: base groups + LoRA-expand tails -----
            def base_mms(tg):
                ps_o = [
                    ps.tile([128, O_SH], F32, name=f"ps_o{tg}_{t}", tag=f"pso{t}")
                    for t in range(4)
                ]
                xs8 = stream.tile([128, KBF, 512], F8, name="xs8", tag="xs8",
                                  bufs=2)
                xsb = stream.tile([128, KBB, 512], BF16, name="xsb", tag="xsb",
                                  bufs=2)
                # fp8 chunks first (consumed first), each paired with its
                # weight chunk on tg==0 so data lands in consumption order
                for c in range(2):
                    nc.sync.dma_start(
                        xs8[:, c * 8:(c + 1) * 8, :],
                        x8Tr[:, c * 8:(c + 1) * 8, tg * 512:(tg + 1) * 512],
                    )
                    if tg == 0:
                        nc.sync.dma_start(
                            wt8[:, c * 8:(c + 1) * 8, :],
                            w8Tr[:, c * 8:(c + 1) * 8, :],
                        )
                for c in range(4):
                    nc.sync.dma_start(
                        xsb[:, c * 4:(c + 1) * 4, :],
                        xbTr[:, c * 4:(c + 1) * 4, tg * 512:(tg + 1) * 512],
                    )
                    if tg == 0:
                        nc.sync.dma_start(
                            wtb[:, c * 4:(c + 1) * 4, :],
                            wbTr[:, c * 4:(c + 1) * 4, :],
                        )
                for kp in range(KBF // 2):
                    for tt in range(4):
                        nc.tensor.matmul(
                            ps_o[tt][:],
                            xs8[:, 2 * kp:2 * kp + 2, tt * 128:(tt + 1) * 128],
                            wt8[:, 2 * kp:2 * kp + 2, :],
                            start=(kp == 0),
                            stop=False,
                            perf_mode=DR,
                        )
                for kb in range(KBB):
                    for tt in range(4):
                        nc.tensor.matmul(
                            ps_o[tt][:],
                            xsb[:, kb, tt * 128:(tt + 1) * 128],
                            wtb[:, kb, :],
                            start=False,
                            stop=(kb == KBB - 1),
                        )
                for tt in range(4):
                    nc.vector.scalar_tensor_tensor(
                        base_sb[:, (tg * 4 + tt) * O_SH:(tg * 4 + tt + 1) * O_SH],
                        ps_o[tt][:],
                        INV_BASE,
                        bias_bc[:],
                        op0=mybir.AluOpType.mult,
                        op1=mybir.AluOpType.add,
                    )

            def tail(tg):
                ps_d = [
                    ps.tile([128, O_SH], F32, name=f"ps_d{tg}_{t}", tag=f"pso{t}")
                    for t in range(4)
                ]
                # whole-group s_masked in two 128KB DMAs (one per source
                # core), prefetched as soon as the AllGather lands
                st = stream.tile([128, 4, 512], F8, name="st", tag="st", bufs=4)
                for h in range(2):
                    nc.gpsimd.dma_start(
                        st[:, :, h * 256:(h + 1) * 256],
                        cc_out[2 * tg + h, :, :].rearrange(
                            "(db p) t -> p db t", p=128),
                    )
                for dbp in range(2):
                    for tt in range(4):
                        nc.tensor.matmul(
                            ps_d[tt][:],
                            st[:, dbp * 2:dbp * 2 + 2, tt * 128:(tt + 1) * 128],
                            bt_all[:, dbp * 2:dbp * 2 + 2, :],
                            start=(dbp == 0),
                            stop=(dbp == 1),
                            perf_mode=DR,
                        )
                ot = stream.tile([128, 4 * O_SH], F32, name="ot", tag="ot", bufs=2)
                for tt in range(4):
                    nc.vector.scalar_tensor_tensor(
                        ot[:, tt * O_SH:(tt + 1) * O_SH],
                        ps_d[tt][:],
                        INV_BASE,
                        base_sb[:, (tg * 4 + tt) * O_SH:(tg * 4 + tt + 1) * O_SH],
                        op0=mybir.AluOpType.mult,
                        op1=mybir.AluOpType.add,
                    )
                    # per-128-token-block store so the last group's copy-out
                    # pipelines with its DMA
                    nc.sync.dma_start(
                        out[tg * 512 + tt * 128:tg * 512 + (tt + 1) * 128, :],
                        ot[:, tt * O_SH:(tt + 1) * O_SH],
                    )

            base_mms(0)
            nc.scalar.dma_start(bt_all[:], bTr[:])
            base_mms(1)
            base_mms(2)
            base_mms(3)
            tail(0)
            tail(1)
            tail(2)
            tail(3)
    return nc


_NC_CACHE = None


def build_in_maps(x, weight, bias, lora_a, lora_b, indices):
    bf = ml_dtypes.bfloat16
    f8 = mybir.dt.np(F8)

    # [128 h-partitions, kb, free] PE-tile-major layouts
    xs = (x * X_SCALE).T.reshape(KB, 128, T)                        # h-major
    x8Tr = np.ascontiguousarray(
        xs[:KBF].astype(f8).transpose(1, 0, 2))                     # (128,KBF,T)
    xbTr = np.ascontiguousarray(
        xs[KBF:].astype(bf).transpose(1, 0, 2))                     # (128,KBB,T)
    aTr = np.ascontiguousarray(
        (lora_a * A_SCALE).astype(f8).reshape(LR, H).T.reshape(KB, 128, LR)
        .transpose(1, 0, 2))                                        # (128,KB,LR)
    idx_f = np.asarray(indices).astype(np.float32)                  # (T,)
    lrow = np.broadcast_to(
        (np.arange(128)[:, None] // 16).astype(np.float32), (128, 4)
    ).copy()
    lrow = lrow + (np.arange(4)[None, :] * 8).astype(np.float32)    # (128, 4)

    in_maps = []
    for c in range(N_CORES):
        ws = (weight[c * O_SH:(c + 1) * O_SH, :] * W_SCALE).T \
            .reshape(KB, 128, O_SH)                                 # h-major
        w8Tc = np.ascontiguousarray(ws[:KBF].astype(f8).transpose(1, 0, 2))
        wbTc = np.ascontiguousarray(ws[KBF:].astype(bf).transpose(1, 0, 2))
        bTc = np.ascontiguousarray(
            (lora_b[:, c * O_SH:(c + 1) * O_SH, :] * B_SCALE).astype(f8)
            .transpose(0, 2, 1).reshape(LR, O_SH)                   # ((l,r), o)
            .reshape(4, 128, O_SH).transpose(1, 0, 2))              # (128,4,O_SH)
        bias_c = np.ascontiguousarray(
            bias[c * O_SH:(c + 1) * O_SH].astype(bf))[None, :]
        idx_bc = np.broadcast_to(
            idx_f[c * T_LOC:(c + 1) * T_LOC][None, :], (128, T_LOC)
        ).copy()
        xl_c = np.ascontiguousarray(
            x[c * T_LOC:(c + 1) * T_LOC, :].astype(f8).T
            .reshape(KB, 128, T_LOC).transpose(1, 0, 2))            # (128,KB,T_LOC)
        in_maps.append({
            "x8Tr": x8Tr, "xbTr": xbTr, "w8Tr": w8Tc, "wbTr": wbTc,
            "xl_r": xl_c, "aTr": aTr, "bTr": bTc,
            "bias_row": bias_c, "idx_bc": idx_bc, "lrow": lrow,
        })
    return in_maps


def kernel(x, weight, bias, lora_a, lora_b, indices):
    global _NC_CACHE
    in_maps = build_in_maps(x, weight, bias, lora_a, lora_b, indices)
    if _NC_CACHE is None:
        _NC_CACHE = _build()
    r = run_bass_kernel_spmd(_NC_CACHE, in_maps, core_ids=list(range(N_CORES)))
    return np.concatenate([r.results[c]["out"] for c in range(N_CORES)], axis=1)


# revision 12
# speedup vs baseline: 1.0115x; 1.0115x over previous
"""ColumnParallelLinear + per-token LoRA (punica add_lora) on 8 NeuronCores.

out = x @ W^T + b + B[idx] @ (A[idx] @ x^T), idx==-1 skips LoRA.

Sharding: tensor-parallel over the output dim (vLLM ColumnParallelLinear):
weight, bias and lora_b are sharded 512-wide per core; lora_a and indices
are replicated. The per-token LoRA shrink (s = A @ x) is sharded over
tokens (256/core) and shared via an on-chip AllGather; the LoRA expand is
folded in as a dense matmul against the routing-masked shrink
(s_masked[t, (l,r)] = (idx[t]==l) * s[t, (l,r)]), so no gather/scatter of
B is ever needed.

Perf notes (measured on HW):
- The kernel is PE-throughput bound: the trace shows matmuls issuing
  back-to-back at the power-throttled clock (HAM k=13/16 ~ 1.95 GHz).
  So v2 cuts PE cycles with fp8e4m3 DoubleRow matmuls (2 MACs/cell/cyc):
  * base matmul: hybrid split — kb blocks 0..15 of the H=4096 contraction
    run as fp8 DoubleRow pairs, blocks 16..31 stay bf16. Full-fp8 misses
    the 2e-2 gate (rel 0.024 measured vs reference); the 16/32 hybrid
    lands at rel ~0.018.
  * LoRA expand: full fp8 DoubleRow (error contribution negligible).
- Scales keep every fp8 operand in e4m3's sweet spot AND make the fp8 and
  bf16 partial products accumulate at a common scale: x ships as 16*x
  (both halves), W as 64*W, A as 16*A, B as 64*B. Base psum = 1024*x@W^T,
  descaled by the copy-out scalar_tensor_tensor (mult 2^-10, add bias).
  Shrink psum = 16*s; expand psum = 1024*delta.
- Inputs stream in a few large chunked DMAs from host-side PE-tile-major
  layouts ([128 partitions, kb, free]), spread across the sync/scalar/
  vector queues (one queue runs its transfers serially at ~140 GB/s).
- The shrink also runs DoubleRow (pairs over kb), and goes first: its
  3.1MB working set is the smallest, and the AllGather (measured ~35us
  on this axon/ncfw path for 128KB/rank) gets the whole base phase to
  complete before tail(0) consumes it two groups later.
- LoRA-expand tails load each group's s_masked with two 128KB DMAs
  (prefetched 4 tails deep) and store per-128-token-block so the last
  group's copy-out pipelines with its DMA.
"""
import json

import numpy as np
import ml_dtypes

import concourse.bass as bass
import concourse.mybir as mybir
import concourse.tile as tile
from concourse.bass_utils import run_bass_kernel_spmd

T, H, O, L, R = 2048, 4096, 4096, 32, 16
N_CORES = 8
O_SH = O // N_CORES          # 512  output cols per core
T_LOC = T // N_CORES         # 256  tokens whose LoRA-shrink this core computes
KB = H // 128                # 32   contraction blocks
KBF = 16                     # kb blocks 0..KBF-1 run fp8 DoubleRow
KBB = KB - KBF               # kb blocks KBF..KB-1 stay bf16
LR = L * R                   # 512  stacked (lora, rank) rows
BF16 = mybir.dt.bfloat16
F32 = mybir.dt.float32
F8 = mybir.dt.float8e4
DR = mybir.MatmulPerfMode.DoubleRow
X_SCALE = 16.0               # x ships as 16*x (fp8 and bf16 halves)
W_SCALE = 64.0               # weight ships as 64*W
A_SCALE = 16.0               # lora_a ships as 16*A  -> shrink psum = 16*s
B_SCALE = 64.0               # lora_b ships as 64*B  -> expand psum = 1024*delta
INV_BASE = 1.0 / (X_SCALE * W_SCALE)


def _split_waits(raw: bytes) -> bytes:
    """This walrus build rejects instructions carrying more than one sync
    wait ("Too many sync wait commands"), but Tile attaches one wait per
    producing proc. Hoist all but one wait of each instruction onto
    single-wait NoOps inserted just before it on the same engine — the
    engine executes its stream in order, so the gating is identical."""
    m = json.loads(raw)
    ctr = 0
    for f in m["functions"]:
        for b in f["blocks"]:
            out = []
            for inst in b["instructions"]:
                si = inst.get("sync_info")
                waits = si.get("on_wait") if si else None
                if waits and len(waits) > 1:
                    for w in waits[:-1]:
                        ctr += 1
                        out.append({
                            "debug": inst.get("debug", 0),
                            "engine": inst["engine"],
                            "ins": [],
                            "name": f"I-wsplit-{ctr}",
                            "opcode": "NoOp",
                            "outs": [],
                            "sync_info": {"on_update": [], "on_wait": [w]},
                        })
                    si["on_wait"] = [waits[-1]]
                out.append(inst)
            b["instructions"] = out
    return json.dumps(m).encode()


class _WaitSplitBass(bass.Bass):
    def to_json_bytes(self) -> bytes:
        return _split_waits(super().to_json_bytes())


def _build() -> bass.Bass:
    nc = _WaitSplitBass()
    # all streamed inputs are PE-tile-major: [128 h-partitions, kb, free]
    x8Tr = nc.dram_tensor("x8Tr", [128, KBF, T], F8, kind="ExternalInput")
    xbTr = nc.dram_tensor("xbTr", [128, KBB, T], BF16, kind="ExternalInput")
    w8Tr = nc.dram_tensor("w8Tr", [128, KBF, O_SH], F8, kind="ExternalInput")
    wbTr = nc.dram_tensor("wbTr", [128, KBB, O_SH], BF16, kind="ExternalInput")
    xl_r = nc.dram_tensor("xl_r", [128, KB, T_LOC], F8, kind="ExternalInput")
    aTr = nc.dram_tensor("aTr", [128, KB, LR], F8, kind="ExternalInput")
    bTr = nc.dram_tensor("bTr", [128, 4, O_SH], F8, kind="ExternalInput")
    bias_row = nc.dram_tensor("bias_row", [1, O_SH], BF16, kind="ExternalInput")
    idx_bc_d = nc.dram_tensor("idx_bc", [128, T_LOC], F32, kind="ExternalInput")
    lrow_d = nc.dram_tensor("lrow", [128, 4], F32, kind="ExternalInput")
    out = nc.dram_tensor("out", [T, O_SH], F32, kind="ExternalOutput")

    with tile.TileContext(nc) as tc:
        with (
            tc.tile_pool(name="res", bufs=1) as res,          # long-lived SBUF
            tc.tile_pool(name="stream", bufs=4) as stream,    # streamed SBUF
            tc.tile_pool(name="ps", bufs=2, space="PSUM") as ps,
            tc.tile_pool(name="dram", bufs=1, space="DRAM") as dram,
        ):
            # ---------------- resident weight tiles ------------------------
            wt8 = res.tile([128, KBF, O_SH], F8, name="wt8")
            wtb = res.tile([128, KBB, O_SH], BF16, name="wtb")
            at_all = res.tile([128, KB, LR], F8, name="at_all")
            xl_all = res.tile([128, KB, T_LOC], F8, name="xl_all")
            bt_all = res.tile([128, 4, O_SH], F8, name="bt_all")
            base_sb = res.tile([128, 16 * O_SH], F32, name="base_sb")

            # bias broadcast [128, 512] f32 via K=1 ones-matmul; the psum
            # descale happens in the same copy-out op later.
            bias_r = res.tile([1, O_SH], BF16, name="bias_r")
            nc.scalar.dma_start(bias_r[:], bias_row[:])
            ones_t = res.tile([1, 128], BF16, name="ones_t")
            nc.vector.memset(ones_t[:], 1.0)

            # ---------------- Phase A: LoRA shrink for local tokens --------
            # The first ~16us are aggregate-HBM-BW bound (~250 GB/s across
            # all queues), so the startup order just keeps arrival aligned
            # with consumption: at on scalar, xl on gpsimd, base streams on
            # sync behind them.
            for c in range(8):
                kc = KB // 8
                nc.scalar.dma_start(
                    at_all[:, c * kc:(c + 1) * kc, :],
                    aTr[:, c * kc:(c + 1) * kc, :],
                )
                nc.gpsimd.dma_start(
                    xl_all[:, c * kc:(c + 1) * kc, :],
                    xl_r[:, c * kc:(c + 1) * kc, :],
                )
            idx_bc = res.tile([128, T_LOC], F32, name="idx_bc_t")
            nc.scalar.dma_start(idx_bc[:], idx_bc_d[:])
            lrow = res.tile([128, 4], F32, name="lrow_t")
            nc.scalar.dma_start(lrow[:], lrow_d[:])

            bias_ps = ps.tile([128, O_SH], F32, name="bias_ps", tag="pso0")
            nc.tensor.matmul(bias_ps[:], ones_t[:], bias_r[:], start=True,
                             stop=True)
            bias_bc = res.tile([128, O_SH], F32, name="bias_bc")
            nc.vector.tensor_copy(bias_bc[:], bias_ps[:])

            ps_s = [ps.tile([128, T_LOC], F32, name=f"ps_s{m}", tag=f"pso{m}")
                    for m in range(4)]
            for kp in range(KB // 2):
                for m in range(4):
                    nc.tensor.matmul(
                        ps_s[m][:],
                        at_all[:, 2 * kp:2 * kp + 2, m * 128:(m + 1) * 128],
                        xl_all[:, 2 * kp:2 * kp + 2, :],
                        start=(kp == 0),
                        stop=(kp == KB // 2 - 1),
                        perf_mode=DR,
                    )

            # routing mask + fp8 downcast, fused: sm = (idx==l(p)) * 16*s
            cc_in = dram.tile([LR, T_LOC], F8, name="cc_in")
            sm = stream.tile([128, 4 * T_LOC], F8, name="sm", tag="sm")
            for m in range(4):
                nc.vector.scalar_tensor_tensor(
                    sm[:, m * T_LOC:(m + 1) * T_LOC],
                    idx_bc[:],
                    lrow[:, m:m + 1],
                    ps_s[m][:],
                    op0=mybir.AluOpType.is_equal,
                    op1=mybir.AluOpType.mult,
                )
            nc.gpsimd.dma_start(
                cc_in[:].rearrange("(m p) t -> p m t", p=128),
                sm[:].rearrange("p (m t) -> p m t", t=T_LOC),
            )

            cc_out = dram.tile([N_CORES, LR, T_LOC], F8, name="cc_out",
                               addr_space="Shared")
            nc.gpsimd.collective_compute(
                "AllGather",
                mybir.AluOpType.bypass,
                replica_groups=[list(range(N_CORES))],
                ins=[cc_in.opt()],
                outs=[cc_out.opt()],
            )

            # ---------------- Phase B: base groups + LoRA-expand tails -----
            def base_mms(tg):
                ps_o = [
                    ps.tile([128, O_SH], F32, name=f"ps_o{tg}_{t}", tag=f"pso{t}")
                    for t in range(4)
                ]
                xs8 = stream.tile([128, KBF, 512], F8, name="xs8", tag="xs8",
                                  bufs=2)
                xsb = stream.tile([128, KBB, 512], BF16, name="xsb", tag="xsb",
                                  bufs=2)
                # fp8 chunks first (consumed first), each paired with its
                # weight chunk on tg==0 so data lands in consumption order
                for c in range(2):
                    nc.sync.dma_start(
                        xs8[:, c * 8:(c + 1) * 8, :],
                        x8Tr[:, c * 8:(c + 1) * 8, tg * 512:(tg + 1) * 512],
                    )
                    if tg == 0:
                        nc.sync.dma_start(
                            wt8[:, c * 8:(c + 1) * 8, :],
                            w8Tr[:, c * 8:(c + 1) * 8, :],
                        )
                for c in range(4):
                    nc.sync.dma_start(
                        xsb[:, c * 4:(c + 1) * 4, :],
                        xbTr[:, c * 4:(c + 1) * 4, tg * 512:(tg + 1) * 512],
                    )
                    if tg == 0:
                        nc.sync.dma_start(
                            wtb[:, c * 4:(c + 1) * 4, :],
                            wbTr[:, c * 4:(c + 1) * 4, :],
                        )
                for kp in range(KBF // 2):
                    for tt in range(4):
                        nc.tensor.matmul(
                            ps_o[tt][:],
                            xs8[:, 2 * kp:2 * kp + 2, tt * 128:(tt + 1) * 128],
                            wt8[:, 2 * kp:2 * kp + 2, :],
                            start=(kp == 0),
                            stop=False,
                            perf_mode=DR,
                        )
                for kb in range(KBB):
                    for tt in range(4):
                        nc.tensor.matmul(
                            ps_o[tt][:],
                            xsb[:, kb, tt * 128:(tt + 1) * 128],
                            wtb[:, kb, :],
                            start=False,
                            stop=(kb == KBB - 1),
                        )
                for tt in range(4):
                    nc.vector.scalar_tensor_tensor(
                        base_sb[:, (tg * 4 + tt) * O_SH:(tg * 4 + tt + 1) * O_SH],
                        ps_o[tt][:],
                        INV_BASE,
                        bias_bc[:],
                        op0=mybir.AluOpType.mult,
                        op1=mybir.AluOpType.add,
                    )

            def tail(tg):
                ps_d = [
                    ps.tile([128, O_SH], F32, name=f"ps_d{tg}_{t}", tag=f"pso{t}")
                    for t in range(4)
                ]
                # whole-group s_masked in two 128KB DMAs (one per source
                # core), prefetched as soon as the AllGather lands
                st = stream.tile([128, 4, 512], F8, name="st", tag="st", bufs=4)
                for h in range(2):
                    nc.gpsimd.dma_start(
                        st[:, :, h * 256:(h + 1) * 256],
                        cc_out[2 * tg + h, :, :].rearrange(
                            "(db p) t -> p db t", p=128),
                    )
                for dbp in range(2):
                    for tt in range(4):
                        nc.tensor.matmul(
                            ps_d[tt][:],
                            st[:, dbp * 2:dbp * 2 + 2, tt * 128:(tt + 1) * 128],
                            bt_all[:, dbp * 2:dbp * 2 + 2, :],
                            start=(dbp == 0),
                            stop=(dbp == 1),
                            perf_mode=DR,
                        )
                ot = stream.tile([128, 4 * O_SH], F32, name="ot", tag="ot", bufs=2)
                for tt in range(4):
                    nc.vector.scalar_tensor_tensor(
                        ot[:, tt * O_SH:(tt + 1) * O_SH],
                        ps_d[tt][:],
                        INV_BASE,
                        base_sb[:, (tg * 4 + tt) * O_SH:(tg * 4 + tt + 1) * O_SH],
                        op0=mybir.AluOpType.mult,
                        op1=mybir.AluOpType.add,
                    )
                    # per-128-token-block store so the last group's copy-out
                    # pipelines with its DMA
                    nc.sync.dma_start(
                        out[tg * 512 + tt * 128:tg * 512 + (tt + 1) * 128, :],
                        ot[:, tt * O_SH:(tt + 1) * O_SH],
                    )

            base_mms(0)
            nc.scalar.dma_start(bt_all[:], bTr[:])
            base_mms(1)
            base_mms(2)
            base_mms(3)
            tail(0)
            tail(1)
            tail(2)
            tail(3)
    return nc


_NC_CACHE = None


def build_in_maps(x, weight, bias, lora_a, lora_b, indices):
    bf = ml_dtypes.bfloat16
    f8 = mybir.dt.np(F8)

    # [128 h-partitions, kb, free] PE-tile-major layouts
    xs = (x * X_SCALE).T.reshape(KB, 128, T)                        # h-major
    x8Tr = np.ascontiguousarray(
        xs[:KBF].astype(f8).transpose(1, 0, 2))                     # (128,KBF,T)
    xbTr = np.ascontiguousarray(
        xs[KBF:].astype(bf).transpose(1, 0, 2))                     # (128,KBB,T)
    aTr = np.ascontiguousarray(
        (lora_a * A_SCALE).astype(f8).reshape(LR, H).T.reshape(KB, 128, LR)
        .transpose(1, 0, 2))                                        # (128,KB,LR)
    idx_f = np.asarray(indices).astype(np.float32)                  # (T,)
    lrow = np.broadcast_to(
        (np.arange(128)[:, None] // 16).astype(np.float32), (128, 4)
    ).copy()
    lrow = lrow + (np.arange(4)[None, :] * 8).astype(np.float32)    # (128, 4)

    in_maps = []
    for c in range(N_CORES):
        ws = (weight[c * O_SH:(c + 1) * O_SH, :] * W_SCALE).T \
            .reshape(KB, 128, O_SH)                                 # h-major
        w8Tc = np.ascontiguousarray(ws[:KBF].astype(f8).transpose(1, 0, 2))
        wbTc = np.ascontiguousarray(ws[KBF:].astype(bf).transpose(1, 0, 2))
        bTc = np.ascontiguousarray(
            (lora_b[:, c * O_SH:(c + 1) * O_SH, :] * B_SCALE).astype(f8)
            .transpose(0, 2, 1).reshape(LR, O_SH)                   # ((l,r), o)
            .reshape(4, 128, O_SH).transpose(1, 0, 2))              # (128,4,O_SH)
        bias_c = np.ascontiguousarray(
            bias[c * O_SH:(c + 1) * O_SH].astype(bf))[None, :]
        idx_bc = np.broadcast_to(
            idx_f[c * T_LOC:(c + 1) * T_LOC][None, :], (128, T_LOC)
        ).copy()
        xl_c = np.ascontiguousarray(
            x[c * T_LOC:(c + 1) * T_LOC, :].astype(f8).T
            .reshape(KB, 128, T_LOC).transpose(1, 0, 2))            # (128,KB,T_LOC)
        in_maps.append({
            "x8Tr": x8Tr, "xbTr": xbTr, "w8Tr": w8Tc, "wbTr": wbTc,
            "xl_r": xl_c, "aTr": aTr, "bTr": bTc,
            "bias_row": bias_c, "idx_bc": idx_bc, "lrow": lrow,
        })
    return in_maps


def kernel(x, weight, bias, lora_a, lora_b, indices):
    global _NC_CACHE
    in_maps = build_in_maps(x, weight, bias, lora_a, lora_b, indices)
    if _NC_CACHE is None:
        _NC_CACHE = _build()
    r = run_bass_kernel_spmd(_NC_CACHE, in_maps, core_ids=list(range(N_CORES)))
    return np.concatenate([r.results[c]["out"] for c in range(N_CORES)], axis=1)


# revision 13
# speedup vs baseline: 1.0203x; 1.0088x over previous
"""ColumnParallelLinear + per-token LoRA (punica add_lora) on 8 NeuronCores.

out = x @ W^T + b + B[idx] @ (A[idx] @ x^T), idx==-1 skips LoRA.

Sharding: tensor-parallel over the output dim (vLLM ColumnParallelLinear):
weight, bias and lora_b are sharded 512-wide per core; lora_a and indices
are replicated. The per-token LoRA shrink (s = A @ x) is sharded over
tokens (256/core) and shared via an on-chip AllGather; the LoRA expand is
folded in as a dense matmul against the routing-masked shrink
(s_masked[t, (l,r)] = (idx[t]==l) * s[t, (l,r)]), so no gather/scatter of
B is ever needed.

Perf notes (measured on HW):
- The kernel is PE-throughput bound: the trace shows matmuls issuing
  back-to-back at the power-throttled clock (HAM k=13/16 ~ 1.95 GHz).
  So v2 cuts PE cycles with fp8e4m3 DoubleRow matmuls (2 MACs/cell/cyc):
  * base matmul: hybrid split — kb blocks 0..15 of the H=4096 contraction
    run as fp8 DoubleRow pairs, blocks 16..31 stay bf16. Full-fp8 misses
    the 2e-2 gate (rel 0.024 measured vs reference); the 16/32 hybrid
    lands at rel ~0.018.
  * LoRA expand: full fp8 DoubleRow (error contribution negligible).
- Scales keep every fp8 operand in e4m3's sweet spot AND make the fp8 and
  bf16 partial products accumulate at a common scale: x ships as 16*x
  (both halves), W as 64*W, A as 16*A, B as 64*B. Base psum = 1024*x@W^T,
  descaled by the copy-out scalar_tensor_tensor (mult 2^-10, add bias).
  Shrink psum = 16*s; expand psum = 1024*delta.
- Inputs stream in a few large chunked DMAs from host-side PE-tile-major
  layouts ([128 partitions, kb, free]), spread across the sync/scalar/
  vector queues (one queue runs its transfers serially at ~140 GB/s).
- The shrink also runs DoubleRow (pairs over kb), and goes first: its
  3.1MB working set is the smallest, and the AllGather (measured ~35us
  on this axon/ncfw path for 128KB/rank) gets the whole base phase to
  complete before tail(0) consumes it two groups later.
- LoRA-expand tails load each group's s_masked with two 128KB DMAs
  (prefetched 4 tails deep) and store per-128-token-block so the last
  group's copy-out pipelines with its DMA.
"""
import json

import numpy as np
import ml_dtypes

import concourse.bass as bass
import concourse.mybir as mybir
import concourse.tile as tile
from concourse.bass_utils import run_bass_kernel_spmd

T, H, O, L, R = 2048, 4096, 4096, 32, 16
N_CORES = 8
O_SH = O // N_CORES          # 512  output cols per core
T_LOC = T // N_CORES         # 256  tokens whose LoRA-shrink this core computes
KB = H // 128                # 32   contraction blocks
KBF = 16                     # kb blocks 0..KBF-1 run fp8 DoubleRow
KBB = KB - KBF               # kb blocks KBF..KB-1 stay bf16
LR = L * R                   # 512  stacked (lora, rank) rows
BF16 = mybir.dt.bfloat16
F32 = mybir.dt.float32
F8 = mybir.dt.float8e4
DR = mybir.MatmulPerfMode.DoubleRow
X_SCALE = 16.0               # x ships as 16*x (fp8 and bf16 halves)
W_SCALE = 64.0               # weight ships as 64*W
A_SCALE = 16.0               # lora_a ships as 16*A  -> shrink psum = 16*s
B_SCALE = 64.0               # lora_b ships as 64*B  -> expand psum = 1024*delta
INV_BASE = 1.0 / (X_SCALE * W_SCALE)


def _split_waits(raw: bytes) -> bytes:
    """This walrus build rejects instructions carrying more than one sync
    wait ("Too many sync wait commands"), but Tile attaches one wait per
    producing proc. Hoist all but one wait of each instruction onto
    single-wait NoOps inserted just before it on the same engine — the
    engine executes its stream in order, so the gating is identical."""
    m = json.loads(raw)
    ctr = 0
    for f in m["functions"]:
        for b in f["blocks"]:
            out = []
            for inst in b["instructions"]:
                si = inst.get("sync_info")
                waits = si.get("on_wait") if si else None
                if waits and len(waits) > 1:
                    for w in waits[:-1]:
                        ctr += 1
                        out.append({
                            "debug": inst.get("debug", 0),
                            "engine": inst["engine"],
                            "ins": [],
                            "name": f"I-wsplit-{ctr}",
                            "opcode": "NoOp",
                            "outs": [],
                            "sync_info": {"on_update": [], "on_wait": [w]},
                        })
                    si["on_wait"] = [waits[-1]]
                out.append(inst)
            b["instructions"] = out
    return json.dumps(m).encode()


class _WaitSplitBass(bass.Bass):
    def to_json_bytes(self) -> bytes:
        return _split_waits(super().to_json_bytes())


def _build() -> bass.Bass:
    nc = _WaitSplitBass()
    # all streamed inputs are PE-tile-major: [128 h-partitions, kb, free]
    x8Tr = nc.dram_tensor("x8Tr", [128, KBF, T], F8, kind="ExternalInput")
    xbTr = nc.dram_tensor("xbTr", [128, KBB, T], BF16, kind="ExternalInput")
    w8Tr = nc.dram_tensor("w8Tr", [128, KBF, O_SH], F8, kind="ExternalInput")
    wbTr = nc.dram_tensor("wbTr", [128, KBB, O_SH], BF16, kind="ExternalInput")
    xl_r = nc.dram_tensor("xl_r", [128, KB, T_LOC], F8, kind="ExternalInput")
    aTr = nc.dram_tensor("aTr", [128, KB, LR], F8, kind="ExternalInput")
    bTr = nc.dram_tensor("bTr", [128, 4, O_SH], F8, kind="ExternalInput")
    bias_row = nc.dram_tensor("bias_row", [1, O_SH], BF16, kind="ExternalInput")
    idx_bc_d = nc.dram_tensor("idx_bc", [128, T_LOC], F32, kind="ExternalInput")
    lrow_d = nc.dram_tensor("lrow", [128, 4], F32, kind="ExternalInput")
    out = nc.dram_tensor("out", [T, O_SH], F32, kind="ExternalOutput")

    with tile.TileContext(nc) as tc:
        with (
            tc.tile_pool(name="res", bufs=1) as res,          # long-lived SBUF
            tc.tile_pool(name="stream", bufs=4) as stream,    # streamed SBUF
            tc.tile_pool(name="ps", bufs=2, space="PSUM") as ps,
            tc.tile_pool(name="dram", bufs=1, space="DRAM") as dram,
        ):
            # ---------------- resident weight tiles ------------------------
            wt8 = res.tile([128, KBF, O_SH], F8, name="wt8")
            wtb = res.tile([128, KBB, O_SH], BF16, name="wtb")
            at_all = res.tile([128, KB, LR], F8, name="at_all")
            xl_all = res.tile([128, KB, T_LOC], F8, name="xl_all")
            bt_all = res.tile([128, 4, O_SH], F8, name="bt_all")
            base_sb = res.tile([128, 16 * O_SH], F32, name="base_sb")

            # bias broadcast [128, 512] f32 via K=1 ones-matmul; the psum
            # descale happens in the same copy-out op later.
            bias_r = res.tile([1, O_SH], BF16, name="bias_r")
            nc.scalar.dma_start(bias_r[:], bias_row[:])
            ones_t = res.tile([1, 128], BF16, name="ones_t")
            nc.vector.memset(ones_t[:], 1.0)

            # ---------------- Phase A: LoRA shrink for local tokens --------
            # The first ~16us are aggregate-HBM-BW bound (~250 GB/s across
            # all queues), so the startup order just keeps arrival aligned
            # with consumption: at on scalar, xl on gpsimd, base streams on
            # sync behind them.
            for c in range(8):
                kc = KB // 8
                nc.scalar.dma_start(
                    at_all[:, c * kc:(c + 1) * kc, :],
                    aTr[:, c * kc:(c + 1) * kc, :],
                )
                nc.gpsimd.dma_start(
                    xl_all[:, c * kc:(c + 1) * kc, :],
                    xl_r[:, c * kc:(c + 1) * kc, :],
                )
            idx_bc = res.tile([128, T_LOC], F32, name="idx_bc_t")
            nc.scalar.dma_start(idx_bc[:], idx_bc_d[:])
            lrow = res.tile([128, 4], F32, name="lrow_t")
            nc.scalar.dma_start(lrow[:], lrow_d[:])

            bias_ps = ps.tile([128, O_SH], F32, name="bias_ps", tag="pso0")
            nc.tensor.matmul(bias_ps[:], ones_t[:], bias_r[:], start=True,
                             stop=True)
            bias_bc = res.tile([128, O_SH], F32, name="bias_bc")
            nc.vector.tensor_copy(bias_bc[:], bias_ps[:])

            ps_s = [ps.tile([128, T_LOC], F32, name=f"ps_s{m}", tag=f"pso{m}")
                    for m in range(4)]
            for kp in range(KB // 2):
                for m in range(4):
                    nc.tensor.matmul(
                        ps_s[m][:],
                        at_all[:, 2 * kp:2 * kp + 2, m * 128:(m + 1) * 128],
                        xl_all[:, 2 * kp:2 * kp + 2, :],
                        start=(kp == 0),
                        stop=(kp == KB // 2 - 1),
                        perf_mode=DR,
                    )

            # routing mask + fp8 downcast, fused: sm = (idx==l(p)) * 16*s
            cc_in = dram.tile([LR, T_LOC], F8, name="cc_in")
            sm = stream.tile([128, 4 * T_LOC], F8, name="sm", tag="sm")
            for m in range(4):
                nc.vector.scalar_tensor_tensor(
                    sm[:, m * T_LOC:(m + 1) * T_LOC],
                    idx_bc[:],
                    lrow[:, m:m + 1],
                    ps_s[m][:],
                    op0=mybir.AluOpType.is_equal,
                    op1=mybir.AluOpType.mult,
                )
            nc.gpsimd.dma_start(
                cc_in[:].rearrange("(m p) t -> p m t", p=128),
                sm[:].rearrange("p (m t) -> p m t", t=T_LOC),
            )

            cc_out = dram.tile([N_CORES, LR, T_LOC], F8, name="cc_out",
                               addr_space="Shared")
            nc.gpsimd.collective_compute(
                "AllGather",
                mybir.AluOpType.bypass,
                replica_groups=[list(range(N_CORES))],
                ins=[cc_in.opt()],
                outs=[cc_out.opt()],
            )

            # ---------------- Phase B: base groups + LoRA-expand tails -----
            def base_mms(tg):
                ps_o = [
                    ps.tile([128, O_SH], F32, name=f"ps_o{tg}_{t}", tag=f"pso{t}")
                    for t in range(4)
                ]
                xs8 = stream.tile([128, KBF, 512], F8, name="xs8", tag="xs8",
                                  bufs=2)
                xsb = stream.tile([128, KBB, 512], BF16, name="xsb", tag="xsb",
                                  bufs=2)
                # fp8 chunks first (consumed first), each paired with its
                # weight chunk on tg==0 so data lands in consumption order
                for c in range(2):
                    nc.sync.dma_start(
                        xs8[:, c * 8:(c + 1) * 8, :],
                        x8Tr[:, c * 8:(c + 1) * 8, tg * 512:(tg + 1) * 512],
                    )
                    if tg == 0:
                        nc.sync.dma_start(
                            wt8[:, c * 8:(c + 1) * 8, :],
                            w8Tr[:, c * 8:(c + 1) * 8, :],
                        )
                for c in range(4):
                    nc.sync.dma_start(
                        xsb[:, c * 4:(c + 1) * 4, :],
                        xbTr[:, c * 4:(c + 1) * 4, tg * 512:(tg + 1) * 512],
                    )
                    if tg == 0:
                        nc.sync.dma_start(
                            wtb[:, c * 4:(c + 1) * 4, :],
                            wbTr[:, c * 4:(c + 1) * 4, :],
                        )
                for kp in range(KBF // 2):
                    for tt in range(4):
                        nc.tensor.matmul(
                            ps_o[tt][:],
                            xs8[:, 2 * kp:2 * kp + 2, tt * 128:(tt + 1) * 128],
                            wt8[:, 2 * kp:2 * kp + 2, :],
                            start=(kp == 0),
                            stop=False,
                            perf_mode=DR,
                        )
                for kb in range(KBB):
                    for tt in range(4):
                        nc.tensor.matmul(
                            ps_o[tt][:],
                            xsb[:, kb, tt * 128:(tt + 1) * 128],
                            wtb[:, kb, :],
                            start=False,
                            stop=(kb == KBB - 1),
                        )
                for tt in range(4):
                    nc.vector.scalar_tensor_tensor(
                        base_sb[:, (tg * 4 + tt) * O_SH:(tg * 4 + tt + 1) * O_SH],
                        ps_o[tt][:],
                        INV_BASE,
                        bias_bc[:],
                        op0=mybir.AluOpType.mult,
                        op1=mybir.AluOpType.add,
                    )

            def tail(tg):
                ps_d = [
                    ps.tile([128, O_SH], F32, name=f"ps_d{tg}_{t}", tag=f"pso{t}")
                    for t in range(4)
                ]
                # whole-group s_masked in two 128KB DMAs (one per source
                # core), prefetched as soon as the AllGather lands
                st = stream.tile([128, 4, 512], F8, name="st", tag="st", bufs=4)
                for h in range(2):
                    nc.gpsimd.dma_start(
                        st[:, :, h * 256:(h + 1) * 256],
                        cc_out[2 * tg + h, :, :].rearrange(
                            "(db p) t -> p db t", p=128),
                    )
                for dbp in range(2):
                    for tt in range(4):
                        nc.tensor.matmul(
                            ps_d[tt][:],
                            st[:, dbp * 2:dbp * 2 + 2, tt * 128:(tt + 1) * 128],
                            bt_all[:, dbp * 2:dbp * 2 + 2, :],
                            start=(dbp == 0),
                            stop=(dbp == 1),
                            perf_mode=DR,
                        )
                ot = stream.tile([128, 4 * O_SH], F32, name="ot", tag="ot", bufs=3)
                for tt in range(4):
                    nc.vector.scalar_tensor_tensor(
                        ot[:, tt * O_SH:(tt + 1) * O_SH],
                        ps_d[tt][:],
                        INV_BASE,
                        base_sb[:, (tg * 4 + tt) * O_SH:(tg * 4 + tt + 1) * O_SH],
                        op0=mybir.AluOpType.mult,
                        op1=mybir.AluOpType.add,
                    )
                    # per-128-token-block stores, alternating queues, so the
                    # last group's copy-out pipelines with its DMAs
                    seng = nc.sync if tt % 2 == 0 else nc.scalar
                    seng.dma_start(
                        out[tg * 512 + tt * 128:tg * 512 + (tt + 1) * 128, :],
                        ot[:, tt * O_SH:(tt + 1) * O_SH],
                    )

            base_mms(0)
            nc.scalar.dma_start(bt_all[:], bTr[:])
            base_mms(1)
            base_mms(2)
            base_mms(3)
            tail(0)
            tail(1)
            tail(2)
            tail(3)
    return nc


_NC_CACHE = None


def build_in_maps(x, weight, bias, lora_a, lora_b, indices):
    bf = ml_dtypes.bfloat16
    f8 = mybir.dt.np(F8)

    # [128 h-partitions, kb, free] PE-tile-major layouts
    xs = (x * X_SCALE).T.reshape(KB, 128, T)                        # h-major
    x8Tr = np.ascontiguousarray(
        xs[:KBF].astype(f8).transpose(1, 0, 2))                     # (128,KBF,T)
    xbTr = np.ascontiguousarray(
        xs[KBF:].astype(bf).transpose(1, 0, 2))                     # (128,KBB,T)
    aTr = np.ascontiguousarray(
        (lora_a * A_SCALE).astype(f8).reshape(LR, H).T.reshape(KB, 128, LR)
        .transpose(1, 0, 2))                                        # (128,KB,LR)
    idx_f = np.asarray(indices).astype(np.float32)                  # (T,)
    lrow = np.broadcast_to(
        (np.arange(128)[:, None] // 16).astype(np.float32), (128, 4)
    ).copy()
    lrow = lrow + (np.arange(4)[None, :] * 8).astype(np.float32)    # (128, 4)

    in_maps = []
    for c in range(N_CORES):
        ws = (weight[c * O_SH:(c + 1) * O_SH, :] * W_SCALE).T \
            .reshape(KB, 128, O_SH)                                 # h-major
        w8Tc = np.ascontiguousarray(ws[:KBF].astype(f8).transpose(1, 0, 2))
        wbTc = np.ascontiguousarray(ws[KBF:].astype(bf).transpose(1, 0, 2))
        bTc = np.ascontiguousarray(
            (lora_b[:, c * O_SH:(c + 1) * O_SH, :] * B_SCALE).astype(f8)
            .transpose(0, 2, 1).reshape(LR, O_SH)                   # ((l,r), o)
            .reshape(4, 128, O_SH).transpose(1, 0, 2))              # (128,4,O_SH)
        bias_c = np.ascontiguousarray(
            bias[c * O_SH:(c + 1) * O_SH].astype(bf))[None, :]
        idx_bc = np.broadcast_to(
            idx_f[c * T_LOC:(c + 1) * T_LOC][None, :], (128, T_LOC)
        ).copy()
        xl_c = np.ascontiguousarray(
            x[c * T_LOC:(c + 1) * T_LOC, :].astype(f8).T
            .reshape(KB, 128, T_LOC).transpose(1, 0, 2))            # (128,KB,T_LOC)
        in_maps.append({
            "x8Tr": x8Tr, "xbTr": xbTr, "w8Tr": w8Tc, "wbTr": wbTc,
            "xl_r": xl_c, "aTr": aTr, "bTr": bTc,
            "bias_row": bias_c, "idx_bc": idx_bc, "lrow": lrow,
        })
    return in_maps


def kernel(x, weight, bias, lora_a, lora_b, indices):
    global _NC_CACHE
    in_maps = build_in_maps(x, weight, bias, lora_a, lora_b, indices)
    if _NC_CACHE is None:
        _NC_CACHE = _build()
    r = run_bass_kernel_spmd(_NC_CACHE, in_maps, core_ids=list(range(N_CORES)))
    return np.concatenate([r.results[c]["out"] for c in range(N_CORES)], axis=1)


# revision 16
# speedup vs baseline: 1.0300x; 1.0095x over previous
"""ColumnParallelLinear + per-token LoRA (punica add_lora) on 8 NeuronCores.

out = x @ W^T + b + B[idx] @ (A[idx] @ x^T), idx==-1 skips LoRA.

Sharding: tensor-parallel over the output dim (vLLM ColumnParallelLinear):
weight, bias and lora_b are sharded 512-wide per core; lora_a and indices
are replicated. The per-token LoRA shrink (s = A @ x) is sharded over
tokens (256/core) and shared via an on-chip AllGather; the LoRA expand is
folded in as a dense matmul against the routing-masked shrink
(s_masked[t, (l,r)] = (idx[t]==l) * s[t, (l,r)]), so no gather/scatter of
B is ever needed.

Perf notes (measured on HW):
- The kernel is PE-throughput bound: the trace shows matmuls issuing
  back-to-back at the power-throttled clock (HAM k=13/16 ~ 1.95 GHz).
  So v2 cuts PE cycles with fp8e4m3 DoubleRow matmuls (2 MACs/cell/cyc):
  * base matmul: hybrid split — kb blocks 0..15 of the H=4096 contraction
    run as fp8 DoubleRow pairs, blocks 16..31 stay bf16. Full-fp8 misses
    the 2e-2 gate (rel 0.024 measured vs reference); the 16/32 hybrid
    lands at rel ~0.018.
  * LoRA expand: full fp8 DoubleRow (error contribution negligible).
- Scales keep every fp8 operand in e4m3's sweet spot AND make the fp8 and
  bf16 partial products accumulate at a common scale: x ships as 16*x
  (both halves), W as 64*W, A as 16*A, B as 64*B. Base psum = 1024*x@W^T,
  descaled by the copy-out scalar_tensor_tensor (mult 2^-10, add bias).
  Shrink psum = 16*s; expand psum = 1024*delta.
- Inputs stream in a few large chunked DMAs from host-side PE-tile-major
  layouts ([128 partitions, kb, free]), spread across the sync/scalar/
  vector queues (one queue runs its transfers serially at ~140 GB/s).
- The shrink also runs DoubleRow (pairs over kb), and goes first: its
  3.1MB working set is the smallest, and the AllGather (measured ~35us
  on this axon/ncfw path for 128KB/rank) gets the whole base phase to
  complete before tail(0) consumes it two groups later.
- LoRA-expand tails load each group's s_masked with two 128KB DMAs
  (prefetched 4 tails deep) and store per-128-token-block so the last
  group's copy-out pipelines with its DMA.
"""
import json

import numpy as np
import ml_dtypes

import concourse.bass as bass
import concourse.mybir as mybir
import concourse.tile as tile
from concourse.bass_utils import run_bass_kernel_spmd

T, H, O, L, R = 2048, 4096, 4096, 32, 16
N_CORES = 8
O_SH = O // N_CORES          # 512  output cols per core
T_LOC = T // N_CORES         # 256  tokens whose LoRA-shrink this core computes
KB = H // 128                # 32   contraction blocks
KBF = 16                     # kb blocks 0..KBF-1 run fp8 DoubleRow
KBB = KB - KBF               # kb blocks KBF..KB-1 stay bf16
LR = L * R                   # 512  stacked (lora, rank) rows
BF16 = mybir.dt.bfloat16
F32 = mybir.dt.float32
F8 = mybir.dt.float8e4
DR = mybir.MatmulPerfMode.DoubleRow
X_SCALE = 16.0               # x ships as 16*x (fp8 and bf16 halves)
W_SCALE = 64.0               # weight ships as 64*W
A_SCALE = 16.0               # lora_a ships as 16*A  -> shrink psum = 16*s
B_SCALE = 64.0               # lora_b ships as 64*B  -> expand psum = 1024*delta
INV_BASE = 1.0 / (X_SCALE * W_SCALE)


def _split_waits(raw: bytes) -> bytes:
    """This walrus build rejects instructions carrying more than one sync
    wait ("Too many sync wait commands"), but Tile attaches one wait per
    producing proc. Hoist all but one wait of each instruction onto
    single-wait NoOps inserted just before it on the same engine — the
    engine executes its stream in order, so the gating is identical."""
    m = json.loads(raw)
    ctr = 0
    for f in m["functions"]:
        for b in f["blocks"]:
            out = []
            for inst in b["instructions"]:
                si = inst.get("sync_info")
                waits = si.get("on_wait") if si else None
                if waits and len(waits) > 1:
                    for w in waits[:-1]:
                        ctr += 1
                        out.append({
                            "debug": inst.get("debug", 0),
                            "engine": inst["engine"],
                            "ins": [],
                            "name": f"I-wsplit-{ctr}",
                            "opcode": "NoOp",
                            "outs": [],
                            "sync_info": {"on_update": [], "on_wait": [w]},
                        })
                    si["on_wait"] = [waits[-1]]
                out.append(inst)
            b["instructions"] = out
    return json.dumps(m).encode()


class _WaitSplitBass(bass.Bass):
    def to_json_bytes(self) -> bytes:
        return _split_waits(super().to_json_bytes())


def _build() -> bass.Bass:
    nc = _WaitSplitBass()
    # all streamed inputs are PE-tile-major: [128 h-partitions, kb, free]
    x8Tr = nc.dram_tensor("x8Tr", [128, KBF, T], F8, kind="ExternalInput")
    xbTr = nc.dram_tensor("xbTr", [128, KBB, T], BF16, kind="ExternalInput")
    w8Tr = nc.dram_tensor("w8Tr", [128, KBF, O_SH], F8, kind="ExternalInput")
    wbTr = nc.dram_tensor("wbTr", [128, KBB, O_SH], BF16, kind="ExternalInput")
    xl_r = nc.dram_tensor("xl_r", [128, KB, T_LOC], F8, kind="ExternalInput")
    aTr = nc.dram_tensor("aTr", [128, KB, LR], F8, kind="ExternalInput")
    bTr = nc.dram_tensor("bTr", [128, 4, O_SH], F8, kind="ExternalInput")
    bias_row = nc.dram_tensor("bias_row", [1, O_SH], BF16, kind="ExternalInput")
    idx_bc_d = nc.dram_tensor("idx_bc", [128, T_LOC], F32, kind="ExternalInput")
    lrow_d = nc.dram_tensor("lrow", [128, 4], F32, kind="ExternalInput")
    out = nc.dram_tensor("out", [T, O_SH], F32, kind="ExternalOutput")

    with tile.TileContext(nc) as tc:
        with (
            tc.tile_pool(name="res", bufs=1) as res,          # long-lived SBUF
            tc.tile_pool(name="stream", bufs=4) as stream,    # streamed SBUF
            tc.tile_pool(name="ps", bufs=2, space="PSUM") as ps,
            tc.tile_pool(name="dram", bufs=1, space="DRAM") as dram,
        ):
            # ---------------- resident weight tiles ------------------------
            wt8 = res.tile([128, KBF, O_SH], F8, name="wt8")
            wtb = res.tile([128, KBB, O_SH], BF16, name="wtb")
            at_all = res.tile([128, KB, LR], F8, name="at_all")
            xl_all = res.tile([128, KB, T_LOC], F8, name="xl_all")
            bt_all = res.tile([128, 4, O_SH], F8, name="bt_all")
            base_sb = res.tile([128, 16 * O_SH], F32, name="base_sb")

            # bias broadcast [128, 512] f32 via K=1 ones-matmul; the psum
            # descale happens in the same copy-out op later.
            bias_r = res.tile([1, O_SH], BF16, name="bias_r")
            nc.scalar.dma_start(bias_r[:], bias_row[:])
            ones_t = res.tile([1, 128], BF16, name="ones_t")
            nc.vector.memset(ones_t[:], 1.0)

            def base_dmas(tg):
                xs8 = stream.tile([128, KBF, 512], F8, name="xs8", tag="xs8",
                                  bufs=2)
                xsb = stream.tile([128, KBB, 512], BF16, name="xsb", tag="xsb",
                                  bufs=2)
                # fp8 chunks first (consumed first), each paired with its
                # weight chunk on tg==0 so data lands in consumption order
                for c in range(2):
                    nc.sync.dma_start(
                        xs8[:, c * 8:(c + 1) * 8, :],
                        x8Tr[:, c * 8:(c + 1) * 8, tg * 512:(tg + 1) * 512],
                    )
                    if tg == 0:
                        nc.sync.dma_start(
                            wt8[:, c * 8:(c + 1) * 8, :],
                            w8Tr[:, c * 8:(c + 1) * 8, :],
                        )
                for c in range(4):
                    nc.sync.dma_start(
                        xsb[:, c * 4:(c + 1) * 4, :],
                        xbTr[:, c * 4:(c + 1) * 4, tg * 512:(tg + 1) * 512],
                    )
                    if tg == 0:
                        nc.sync.dma_start(
                            wtb[:, c * 4:(c + 1) * 4, :],
                            wbTr[:, c * 4:(c + 1) * 4, :],
                        )
                return xs8, xsb

            def base_dr_mms(tg, ps_o, xs8):
                for kp in range(KBF // 2):
                    for tt in range(4):
                        nc.tensor.matmul(
                            ps_o[tt][:],
                            xs8[:, 2 * kp:2 * kp + 2, tt * 128:(tt + 1) * 128],
                            wt8[:, 2 * kp:2 * kp + 2, :],
                            start=(kp == 0),
                            stop=False,
                            perf_mode=DR,
                        )

            def base_bf_mms(tg, ps_o, xsb):
                for kb in range(KBB):
                    for tt in range(4):
                        nc.tensor.matmul(
                            ps_o[tt][:],
                            xsb[:, kb, tt * 128:(tt + 1) * 128],
                            wtb[:, kb, :],
                            start=False,
                            stop=(kb == KBB - 1),
                        )
                for tt in range(4):
                    nc.vector.scalar_tensor_tensor(
                        base_sb[:, (tg * 4 + tt) * O_SH:(tg * 4 + tt + 1) * O_SH],
                        ps_o[tt][:],
                        INV_BASE,
                        bias_bc[:],
                        op0=mybir.AluOpType.mult,
                        op1=mybir.AluOpType.add,
                    )

            def base_mms(tg):
                ps_o = [
                    ps.tile([128, O_SH], F32, name=f"ps_o{tg}_{t}", tag=f"pso{t}")
                    for t in range(4)
                ]
                xs8, xsb = base_dmas(tg)
                base_dr_mms(tg, ps_o, xs8)
                base_bf_mms(tg, ps_o, xsb)

            # ---------------- Phase A: LoRA shrink for local tokens --------
            # The first ~16us are aggregate-HBM-BW bound (~250 GB/s across
            # all queues), so the startup order just keeps arrival aligned
            # with consumption: at on scalar, xl on gpsimd, base streams on
            # sync behind them.
            for c in range(8):
                kc = KB // 8
                nc.scalar.dma_start(
                    at_all[:, c * kc:(c + 1) * kc, :],
                    aTr[:, c * kc:(c + 1) * kc, :],
                )
                nc.gpsimd.dma_start(
                    xl_all[:, c * kc:(c + 1) * kc, :],
                    xl_r[:, c * kc:(c + 1) * kc, :],
                )
            idx_bc = res.tile([128, T_LOC], F32, name="idx_bc_t")
            nc.scalar.dma_start(idx_bc[:], idx_bc_d[:])
            lrow = res.tile([128, 4], F32, name="lrow_t")
            nc.scalar.dma_start(lrow[:], lrow_d[:])

            bias_ps = ps.tile([128, O_SH], F32, name="bias_ps", tag="pso0")
            nc.tensor.matmul(bias_ps[:], ones_t[:], bias_r[:], start=True,
                             stop=True)
            bias_bc = res.tile([128, O_SH], F32, name="bias_bc")
            nc.vector.tensor_copy(bias_bc[:], bias_ps[:])

            def shrink_mms(kp_lo, kp_hi):
                for kp in range(kp_lo, kp_hi):
                    for m in range(4):
                        nc.tensor.matmul(
                            ps_s[m][:],
                            at_all[:, 2 * kp:2 * kp + 2, m * 128:(m + 1) * 128],
                            xl_all[:, 2 * kp:2 * kp + 2, :],
                            start=(kp == 0),
                            stop=(kp == KB // 2 - 1),
                            perf_mode=DR,
                        )

            ps_s = [ps.tile([128, T_LOC], F32, name=f"ps_s{m}", tag=f"pso{m}")
                    for m in range(4)]
            shrink_mms(0, KB // 4)
            # fill the mid-shrink DMA-wait (the at stream is still arriving)
            # with group 0's fp8 half, whose data landed first on sync
            g0_ps = [ps.tile([128, O_SH], F32, name=f"ps_o0_{t}", tag=f"pso{t}")
                     for t in range(4)]
            g0_xs8, g0_xsb = base_dmas(0)
            base_dr_mms(0, g0_ps, g0_xs8)
            shrink_mms(KB // 4, KB // 2)

            # routing mask + fp8 downcast, fused: sm = (idx==l(p)) * 16*s
            cc_in = dram.tile([LR, T_LOC], F8, name="cc_in")
            sm = stream.tile([128, 4 * T_LOC], F8, name="sm", tag="sm")
            for m in range(4):
                nc.vector.scalar_tensor_tensor(
                    sm[:, m * T_LOC:(m + 1) * T_LOC],
                    idx_bc[:],
                    lrow[:, m:m + 1],
                    ps_s[m][:],
                    op0=mybir.AluOpType.is_equal,
                    op1=mybir.AluOpType.mult,
                )
            nc.gpsimd.dma_start(
                cc_in[:].rearrange("(m p) t -> p m t", p=128),
                sm[:].rearrange("p (m t) -> p m t", t=T_LOC),
            )

            cc_out = dram.tile([N_CORES, LR, T_LOC], F8, name="cc_out",
                               addr_space="Shared")
            nc.gpsimd.collective_compute(
                "AllGather",
                mybir.AluOpType.bypass,
                replica_groups=[list(range(N_CORES))],
                ins=[cc_in.opt()],
                outs=[cc_out.opt()],
            )

            # ---------------- Phase B: base groups + LoRA-expand tails -----
            def tail(tg):
                ps_d = [
                    ps.tile([128, O_SH], F32, name=f"ps_d{tg}_{t}", tag=f"pso{t}")
                    for t in range(4)
                ]
                # whole-group s_masked in two 128KB DMAs (one per source
                # core), prefetched as soon as the AllGather lands
                st = stream.tile([128, 4, 512], F8, name="st", tag="st", bufs=4)
                for h in range(2):
                    nc.gpsimd.dma_start(
                        st[:, :, h * 256:(h + 1) * 256],
                        cc_out[2 * tg + h, :, :].rearrange(
                            "(db p) t -> p db t", p=128),
                    )
                for dbp in range(2):
                    for tt in range(4):
                        nc.tensor.matmul(
                            ps_d[tt][:],
                            st[:, dbp * 2:dbp * 2 + 2, tt * 128:(tt + 1) * 128],
                            bt_all[:, dbp * 2:dbp * 2 + 2, :],
                            start=(dbp == 0),
                            stop=(dbp == 1),
                            perf_mode=DR,
                        )
                ot = stream.tile([128, 4 * O_SH], F32, name="ot", tag="ot", bufs=3)
                for tt in range(4):
                    nc.vector.scalar_tensor_tensor(
                        ot[:, tt * O_SH:(tt + 1) * O_SH],
                        ps_d[tt][:],
                        INV_BASE,
                        base_sb[:, (tg * 4 + tt) * O_SH:(tg * 4 + tt + 1) * O_SH],
                        op0=mybir.AluOpType.mult,
                        op1=mybir.AluOpType.add,
                    )
                    # per-128-token-block stores, alternating queues, so the
                    # last group's copy-out pipelines with its DMAs
                    seng = nc.sync if tt % 2 == 0 else nc.scalar
                    seng.dma_start(
                        out[tg * 512 + tt * 128:tg * 512 + (tt + 1) * 128, :],
                        ot[:, tt * O_SH:(tt + 1) * O_SH],
                    )

            base_bf_mms(0, g0_ps, g0_xsb)
            nc.scalar.dma_start(bt_all[:], bTr[:])
            base_mms(1)
            base_mms(2)
            base_mms(3)
            tail(0)
            tail(1)
            tail(2)
            tail(3)
    return nc


_NC_CACHE = None


def build_in_maps(x, weight, bias, lora_a, lora_b, indices):
    bf = ml_dtypes.bfloat16
    f8 = mybir.dt.np(F8)

    # [128 h-partitions, kb, free] PE-tile-major layouts
    xs = (x * X_SCALE).T.reshape(KB, 128, T)                        # h-major
    x8Tr = np.ascontiguousarray(
        xs[:KBF].astype(f8).transpose(1, 0, 2))                     # (128,KBF,T)
    xbTr = np.ascontiguousarray(
        xs[KBF:].astype(bf).transpose(1, 0, 2))                     # (128,KBB,T)
    aTr = np.ascontiguousarray(
        (lora_a * A_SCALE).astype(f8).reshape(LR, H).T.reshape(KB, 128, LR)
        .transpose(1, 0, 2))                                        # (128,KB,LR)
    idx_f = np.asarray(indices).astype(np.float32)                  # (T,)
    lrow = np.broadcast_to(
        (np.arange(128)[:, None] // 16).astype(np.float32), (128, 4)
    ).copy()
    lrow = lrow + (np.arange(4)[None, :] * 8).astype(np.float32)    # (128, 4)

    in_maps = []
    for c in range(N_CORES):
        ws = (weight[c * O_SH:(c + 1) * O_SH, :] * W_SCALE).T \
            .reshape(KB, 128, O_SH)                                 # h-major
        w8Tc = np.ascontiguousarray(ws[:KBF].astype(f8).transpose(1, 0, 2))
        wbTc = np.ascontiguousarray(ws[KBF:].astype(bf).transpose(1, 0, 2))
        bTc = np.ascontiguousarray(
            (lora_b[:, c * O_SH:(c + 1) * O_SH, :] * B_SCALE).astype(f8)
            .transpose(0, 2, 1).reshape(LR, O_SH)                   # ((l,r), o)
            .reshape(4, 128, O_SH).transpose(1, 0, 2))              # (128,4,O_SH)
        bias_c = np.ascontiguousarray(
            bias[c * O_SH:(c + 1) * O_SH].astype(bf))[None, :]
        idx_bc = np.broadcast_to(
            idx_f[c * T_LOC:(c + 1) * T_LOC][None, :], (128, T_LOC)
        ).copy()
        xl_c = np.ascontiguousarray(
            x[c * T_LOC:(c + 1) * T_LOC, :].astype(f8).T
            .reshape(KB, 128, T_LOC).transpose(1, 0, 2))            # (128,KB,T_LOC)
        in_maps.append({
            "x8Tr": x8Tr, "xbTr": xbTr, "w8Tr": w8Tc, "wbTr": wbTc,
            "xl_r": xl_c, "aTr": aTr, "bTr": bTc,
            "bias_row": bias_c, "idx_bc": idx_bc, "lrow": lrow,
        })
    return in_maps


def kernel(x, weight, bias, lora_a, lora_b, indices):
    global _NC_CACHE
    in_maps = build_in_maps(x, weight, bias, lora_a, lora_b, indices)
    if _NC_CACHE is None:
        _NC_CACHE = _build()
    r = run_bass_kernel_spmd(_NC_CACHE, in_maps, core_ids=list(range(N_CORES)))
    return np.concatenate([r.results[c]["out"] for c in range(N_CORES)], axis=1)


# revision 19
# speedup vs baseline: 1.0820x; 1.0504x over previous
"""ColumnParallelLinear + per-token LoRA (punica add_lora) on 8 NeuronCores.

out = x @ W^T + b + B[idx] @ (A[idx] @ x^T), idx==-1 skips LoRA.

Sharding: tensor-parallel over the output dim (vLLM ColumnParallelLinear):
weight, bias and lora_b are sharded 512-wide per core; lora_a and indices
are replicated. The per-token LoRA shrink (s = A @ x) is sharded over
tokens (256/core) and shared via an on-chip AllGather; the LoRA expand is
folded in as a dense matmul against the routing-masked shrink
(s_masked[t, (l,r)] = (idx[t]==l) * s[t, (l,r)]), so no gather/scatter of
B is ever needed.

Perf notes (measured on HW, ~144-147us vs the 197us bf16 baseline):
- The kernel is PE-throughput bound: the trace shows matmuls issuing
  back-to-back at the power-throttled clock (HAM k=13/16 ~ 1.95 GHz).
  So it cuts PE cycles with fp8e4m3 DoubleRow matmuls (2 MACs/cell/cyc):
  * base matmul: hybrid split — kb blocks 0..15 of the H=4096 contraction
    run as fp8 DoubleRow pairs, blocks 16..31 stay bf16. Full-fp8 misses
    the 2e-2 gate (rel 0.024 measured vs reference); the 16/32 hybrid
    lands at rel ~0.018.
  * LoRA expand: full fp8 DoubleRow (error contribution negligible).
- Scales keep every fp8 operand in e4m3's sweet spot AND make the fp8 and
  bf16 partial products accumulate at a common scale: x ships as 16*x
  (both halves), W as 64*W, A as 16*A, B as 64*B. Base psum = 1024*x@W^T,
  descaled by the copy-out scalar_tensor_tensor (mult 2^-10, add bias).
  Shrink psum = 16*s; expand psum = 1024*delta.
- Inputs stream in a few large chunked DMAs from host-side PE-tile-major
  layouts ([128 partitions, kb, free]), spread across the sync/scalar/
  vector queues (one queue runs its transfers serially at ~140 GB/s).
- The shrink also runs DoubleRow (pairs over kb), and goes first: its
  3.1MB working set is the smallest, and the AllGather (measured ~35us
  on this axon/ncfw path for 128KB/rank) gets the whole base phase to
  complete before tail(0) consumes it two groups later.
- LoRA-expand tails load each group's s_masked with two 128KB DMAs
  (prefetched 4 tails deep) and store per-128-token-block so the last
  group's copy-out pipelines with its DMA.
"""
import json

import numpy as np
import ml_dtypes

import concourse.bass as bass
import concourse.mybir as mybir
import concourse.tile as tile
from concourse.bass_utils import run_bass_kernel_spmd

T, H, O, L, R = 2048, 4096, 4096, 32, 16
N_CORES = 8
O_SH = O // N_CORES          # 512  output cols per core
T_LOC = T // N_CORES         # 256  tokens whose LoRA-shrink this core computes
KB = H // 128                # 32   contraction blocks
KBF = 16                     # kb blocks 0..KBF-1 run fp8 DoubleRow
KBB = KB - KBF               # kb blocks KBF..KB-1 stay bf16
LR = L * R                   # 512  stacked (lora, rank) rows
BF16 = mybir.dt.bfloat16
F32 = mybir.dt.float32
F8 = mybir.dt.float8e4
DR = mybir.MatmulPerfMode.DoubleRow
X_SCALE = 16.0               # x ships as 16*x (fp8 and bf16 halves)
W_SCALE = 64.0               # weight ships as 64*W
A_SCALE = 16.0               # lora_a ships as 16*A  -> shrink psum = 16*s
B_SCALE = 64.0               # lora_b ships as 64*B  -> expand psum = 1024*delta
INV_BASE = 1.0 / (X_SCALE * W_SCALE)


def _split_waits(raw: bytes) -> bytes:
    """This walrus build rejects instructions carrying more than one sync
    wait ("Too many sync wait commands"), but Tile attaches one wait per
    producing proc. Hoist all but one wait of each instruction onto
    single-wait NoOps inserted just before it on the same engine — the
    engine executes its stream in order, so the gating is identical."""
    m = json.loads(raw)
    ctr = 0
    for f in m["functions"]:
        for b in f["blocks"]:
            out = []
            for inst in b["instructions"]:
                si = inst.get("sync_info")
                waits = si.get("on_wait") if si else None
                if waits and len(waits) > 1:
                    for w in waits[:-1]:
                        ctr += 1
                        out.append({
                            "debug": inst.get("debug", 0),
                            "engine": inst["engine"],
                            "ins": [],
                            "name": f"I-wsplit-{ctr}",
                            "opcode": "NoOp",
                            "outs": [],
                            "sync_info": {"on_update": [], "on_wait": [w]},
                        })
                    si["on_wait"] = [waits[-1]]
                out.append(inst)
            b["instructions"] = out
    return json.dumps(m).encode()


class _WaitSplitBass(bass.Bass):
    def to_json_bytes(self) -> bytes:
        return _split_waits(super().to_json_bytes())


def _build() -> bass.Bass:
    nc = _WaitSplitBass()
    # all streamed inputs are PE-tile-major: [128 h-partitions, kb, free]
    x8Tr = nc.dram_tensor("x8Tr", [128, KBF, T], F8, kind="ExternalInput")
    xbTr = nc.dram_tensor("xbTr", [128, KBB, T], BF16, kind="ExternalInput")
    w8Tr = nc.dram_tensor("w8Tr", [128, KBF, O_SH], F8, kind="ExternalInput")
    wbTr = nc.dram_tensor("wbTr", [128, KBB, O_SH], BF16, kind="ExternalInput")
    xl_r = nc.dram_tensor("xl_r", [128, KB, T_LOC], F8, kind="ExternalInput")
    aTr = nc.dram_tensor("aTr", [128, KB, LR], F8, kind="ExternalInput")
    bTr = nc.dram_tensor("bTr", [128, 4, O_SH], F8, kind="ExternalInput")
    bias_row = nc.dram_tensor("bias_row", [128, O_SH], BF16, kind="ExternalInput")
    idx_bc_d = nc.dram_tensor("idx_bc", [128, T_LOC], F32, kind="ExternalInput")
    lrow_d = nc.dram_tensor("lrow", [128, 4], F32, kind="ExternalInput")
    out = nc.dram_tensor("out", [T, O_SH], F32, kind="ExternalOutput")

    with tile.TileContext(nc) as tc:
        with (
            tc.tile_pool(name="res", bufs=1) as res,          # long-lived SBUF
            tc.tile_pool(name="stream", bufs=4) as stream,    # streamed SBUF
            tc.tile_pool(name="ps", bufs=2, space="PSUM") as ps,
            tc.tile_pool(name="dram", bufs=1, space="DRAM") as dram,
        ):
            # ---------------- resident weight tiles ------------------------
            wt8 = res.tile([128, KBF, O_SH], F8, name="wt8")
            wtb = res.tile([128, KBB, O_SH], BF16, name="wtb")
            at_all = res.tile([128, KB, LR], F8, name="at_all")
            xl_all = res.tile([128, KB, T_LOC], F8, name="xl_all")
            bt_all = res.tile([128, 4, O_SH], F8, name="bt_all")
            base_sb = res.tile([128, 16 * O_SH], F32, name="base_sb")


            def base_dmas(tg):
                xs8 = stream.tile([128, KBF, 512], F8, name="xs8", tag="xs8",
                                  bufs=2)
                xsb = stream.tile([128, KBB, 512], BF16, name="xsb", tag="xsb",
                                  bufs=2)
                # fp8 chunks first (consumed first), each paired with its
                # weight chunk on tg==0 so data lands in consumption order
                for c in range(2):
                    nc.sync.dma_start(
                        xs8[:, c * 8:(c + 1) * 8, :],
                        x8Tr[:, c * 8:(c + 1) * 8, tg * 512:(tg + 1) * 512],
                    )
                    if tg == 0:
                        nc.sync.dma_start(
                            wt8[:, c * 8:(c + 1) * 8, :],
                            w8Tr[:, c * 8:(c + 1) * 8, :],
                        )
                for c in range(4):
                    nc.sync.dma_start(
                        xsb[:, c * 4:(c + 1) * 4, :],
                        xbTr[:, c * 4:(c + 1) * 4, tg * 512:(tg + 1) * 512],
                    )
                    if tg == 0:
                        nc.sync.dma_start(
                            wtb[:, c * 4:(c + 1) * 4, :],
                            wbTr[:, c * 4:(c + 1) * 4, :],
                        )
                return xs8, xsb

            def base_dr_mms(tg, ps_o, xs8, kp_lo=0, kp_hi=KBF // 2):
                for kp in range(kp_lo, kp_hi):
                    for tt in range(4):
                        nc.tensor.matmul(
                            ps_o[tt][:],
                            xs8[:, 2 * kp:2 * kp + 2, tt * 128:(tt + 1) * 128],
                            wt8[:, 2 * kp:2 * kp + 2, :],
                            start=(kp == 0),
                            stop=False,
                            perf_mode=DR,
                        )

            def base_bf_mms(tg, ps_o, xsb):
                for kb in range(KBB):
                    for tt in range(4):
                        nc.tensor.matmul(
                            ps_o[tt][:],
                            xsb[:, kb, tt * 128:(tt + 1) * 128],
                            wtb[:, kb, :],
                            start=False,
                            stop=(kb == KBB - 1),
                        )
                for tt in range(4):
                    nc.vector.scalar_tensor_tensor(
                        base_sb[:, (tg * 4 + tt) * O_SH:(tg * 4 + tt + 1) * O_SH],
                        ps_o[tt][:],
                        INV_BASE,
                        bias_bc[:],
                        op0=mybir.AluOpType.mult,
                        op1=mybir.AluOpType.add,
                    )

            def base_mms(tg):
                ps_o = [
                    ps.tile([128, O_SH], F32, name=f"ps_o{tg}_{t}", tag=f"pso{t}")
                    for t in range(4)
                ]
                xs8, xsb = base_dmas(tg)
                base_dr_mms(tg, ps_o, xs8)
                base_bf_mms(tg, ps_o, xsb)

            # ---------------- Phase A: LoRA shrink for local tokens --------
            # The first ~16us are aggregate-HBM-BW bound (~250 GB/s across
            # all queues), so the startup order just keeps arrival aligned
            # with consumption: at on scalar, xl on gpsimd, base streams on
            # sync behind them.
            for c in range(8):
                kc = KB // 8
                nc.scalar.dma_start(
                    at_all[:, c * kc:(c + 1) * kc, :],
                    aTr[:, c * kc:(c + 1) * kc, :],
                )
                nc.gpsimd.dma_start(
                    xl_all[:, c * kc:(c + 1) * kc, :],
                    xl_r[:, c * kc:(c + 1) * kc, :],
                )
            idx_bc = res.tile([128, T_LOC], F32, name="idx_bc_t")
            nc.scalar.dma_start(idx_bc[:], idx_bc_d[:])
            lrow = res.tile([128, 4], F32, name="lrow_t")
            nc.scalar.dma_start(lrow[:], lrow_d[:])
            # host-broadcast bias lands mid-startup on the scalar queue; it is
            # first needed by group 0's copy-out (~45us in)
            bias_bc = res.tile([128, O_SH], BF16, name="bias_bc")
            nc.scalar.dma_start(bias_bc[:], bias_row[:])

            def shrink_mms(kp_lo, kp_hi):
                for kp in range(kp_lo, kp_hi):
                    for m in range(4):
                        nc.tensor.matmul(
                            ps_s[m][:],
                            at_all[:, 2 * kp:2 * kp + 2, m * 128:(m + 1) * 128],
                            xl_all[:, 2 * kp:2 * kp + 2, :],
                            start=(kp == 0),
                            stop=(kp == KB // 2 - 1),
                            perf_mode=DR,
                        )

            ps_s = [ps.tile([128, T_LOC], F32, name=f"ps_s{m}", tag=f"pso{m}")
                    for m in range(4)]
            # fine-interleave the shrink with group 0's fp8 half: whichever
            # input stream (at/xl on scalar+gpsimd vs x8/w8 on sync) has
            # arrived keeps the PE fed during the BW-bound startup
            g0_ps = [ps.tile([128, O_SH], F32, name=f"ps_o0_{t}", tag=f"pso{t}")
                     for t in range(4)]
            g0_xs8, g0_xsb = base_dmas(0)
            for q in range(4):
                shrink_mms(2 * q, 2 * q + 2)
                base_dr_mms(0, g0_ps, g0_xs8, 2 * q, 2 * q + 2)
            shrink_mms(KB // 4, KB // 2)

            # routing mask + fp8 downcast, fused: sm = (idx==l(p)) * 16*s
            cc_in = dram.tile([LR, T_LOC], F8, name="cc_in")
            sm = stream.tile([128, 4 * T_LOC], F8, name="sm", tag="sm")
            for m in range(4):
                nc.vector.scalar_tensor_tensor(
                    sm[:, m * T_LOC:(m + 1) * T_LOC],
                    idx_bc[:],
                    lrow[:, m:m + 1],
                    ps_s[m][:],
                    op0=mybir.AluOpType.is_equal,
                    op1=mybir.AluOpType.mult,
                )
            nc.gpsimd.dma_start(
                cc_in[:].rearrange("(m p) t -> p m t", p=128),
                sm[:].rearrange("p (m t) -> p m t", t=T_LOC),
            )

            cc_out = dram.tile([N_CORES, LR, T_LOC], F8, name="cc_out",
                               addr_space="Shared")
            nc.gpsimd.collective_compute(
                "AllGather",
                mybir.AluOpType.bypass,
                replica_groups=[list(range(N_CORES))],
                ins=[cc_in.opt()],
                outs=[cc_out.opt()],
            )

            # ---------------- Phase B: base groups + LoRA-expand tails -----
            def tail(tg):
                ps_d = [
                    ps.tile([128, O_SH], F32, name=f"ps_d{tg}_{t}", tag=f"pso{t}")
                    for t in range(4)
                ]
                # whole-group s_masked in two 128KB DMAs (one per source
                # core), prefetched as soon as the AllGather lands
                st = stream.tile([128, 4, 512], F8, name="st", tag="st", bufs=4)
                for h in range(2):
                    nc.gpsimd.dma_start(
                        st[:, :, h * 256:(h + 1) * 256],
                        cc_out[2 * tg + h, :, :].rearrange(
                            "(db p) t -> p db t", p=128),
                    )
                for dbp in range(2):
                    for tt in range(4):
                        nc.tensor.matmul(
                            ps_d[tt][:],
                            st[:, dbp * 2:dbp * 2 + 2, tt * 128:(tt + 1) * 128],
                            bt_all[:, dbp * 2:dbp * 2 + 2, :],
                            start=(dbp == 0),
                            stop=(dbp == 1),
                            perf_mode=DR,
                        )
                ot = stream.tile([128, 4 * O_SH], F32, name="ot", tag="ot", bufs=4)
                for tt in range(4):
                    nc.vector.scalar_tensor_tensor(
                        ot[:, tt * O_SH:(tt + 1) * O_SH],
                        ps_d[tt][:],
                        INV_BASE,
                        base_sb[:, (tg * 4 + tt) * O_SH:(tg * 4 + tt + 1) * O_SH],
                        op0=mybir.AluOpType.mult,
                        op1=mybir.AluOpType.add,
                    )
                    # per-128-token-block stores, alternating queues, so the
                    # last group's copy-out pipelines with its DMAs
                    seng = nc.sync if tt % 2 == 0 else nc.scalar
                    seng.dma_start(
                        out[tg * 512 + tt * 128:tg * 512 + (tt + 1) * 128, :],
                        ot[:, tt * O_SH:(tt + 1) * O_SH],
                    )

            base_bf_mms(0, g0_ps, g0_xsb)
            nc.scalar.dma_start(bt_all[:], bTr[:])
            base_mms(1)
            base_mms(2)
            base_mms(3)
            tail(0)
            tail(1)
            tail(2)
            tail(3)
    return nc


_NC_CACHE = None


def build_in_maps(x, weight, bias, lora_a, lora_b, indices):
    bf = ml_dtypes.bfloat16
    f8 = mybir.dt.np(F8)

    # [128 h-partitions, kb, free] PE-tile-major layouts
    xs = (x * X_SCALE).T.reshape(KB, 128, T)                        # h-major
    x8Tr = np.ascontiguousarray(
        xs[:KBF].astype(f8).transpose(1, 0, 2))                     # (128,KBF,T)
    xbTr = np.ascontiguousarray(
        xs[KBF:].astype(bf).transpose(1, 0, 2))                     # (128,KBB,T)
    aTr = np.ascontiguousarray(
        (lora_a * A_SCALE).astype(f8).reshape(LR, H).T.reshape(KB, 128, LR)
        .transpose(1, 0, 2))                                        # (128,KB,LR)
    idx_f = np.asarray(indices).astype(np.float32)                  # (T,)
    lrow = np.broadcast_to(
        (np.arange(128)[:, None] // 16).astype(np.float32), (128, 4)
    ).copy()
    lrow = lrow + (np.arange(4)[None, :] * 8).astype(np.float32)    # (128, 4)

    in_maps = []
    for c in range(N_CORES):
        ws = (weight[c * O_SH:(c + 1) * O_SH, :] * W_SCALE).T \
            .reshape(KB, 128, O_SH)                                 # h-major
        w8Tc = np.ascontiguousarray(ws[:KBF].astype(f8).transpose(1, 0, 2))
        wbTc = np.ascontiguousarray(ws[KBF:].astype(bf).transpose(1, 0, 2))
        bTc = np.ascontiguousarray(
            (lora_b[:, c * O_SH:(c + 1) * O_SH, :] * B_SCALE).astype(f8)
            .transpose(0, 2, 1).reshape(LR, O_SH)                   # ((l,r), o)
            .reshape(4, 128, O_SH).transpose(1, 0, 2))              # (128,4,O_SH)
        bias_c = np.broadcast_to(
            bias[c * O_SH:(c + 1) * O_SH].astype(bf)[None, :],
            (128, O_SH)).copy()
        idx_bc = np.broadcast_to(
            idx_f[c * T_LOC:(c + 1) * T_LOC][None, :], (128, T_LOC)
        ).copy()
        xl_c = np.ascontiguousarray(
            x[c * T_LOC:(c + 1) * T_LOC, :].astype(f8).T
            .reshape(KB, 128, T_LOC).transpose(1, 0, 2))            # (128,KB,T_LOC)
        in_maps.append({
            "x8Tr": x8Tr, "xbTr": xbTr, "w8Tr": w8Tc, "wbTr": wbTc,
            "xl_r": xl_c, "aTr": aTr, "bTr": bTc,
            "bias_row": bias_c, "idx_bc": idx_bc, "lrow": lrow,
        })
    return in_maps


def kernel(x, weight, bias, lora_a, lora_b, indices):
    global _NC_CACHE
    in_maps = build_in_maps(x, weight, bias, lora_a, lora_b, indices)
    if _NC_CACHE is None:
        _NC_CACHE = _build()
    r = run_bass_kernel_spmd(_NC_CACHE, in_maps, core_ids=list(range(N_CORES)))
    return np.concatenate([r.results[c]["out"] for c in range(N_CORES)], axis=1)
